# revision 8
# baseline (speedup 1.0000x reference)
"""Trainium2 Bass kernel for nn_ESBN_77352361001553 (scatter_memory).

Math being computed (see the reference's own faithfulness note): the conv
encoder output is dead code, and the LSTM input is constant zeros, so the
gate pre-activation contribution from the input is the constant bih + bhh
for every step and every batch element. Every batch row therefore follows
the identical 16-step, 512-dim LSTM trajectory from zero state, and the
(16, 1024, 4) output is out_t = Wo @ h_t + bo broadcast across batch.

Sharding: pure data parallelism over the batch dim — each of the 8 cores
owns a 128-wide batch shard. Each core runs the recurrence on-chip:
 - gates matvec on the PE as 64 (LDWEIGHTS, MATMUL N=1) pairs per step in
   fp16 (FWL fast-weight-load path, ~27 ns/pair), accumulating the
   [128, 16] gate columns in PSUM,
 - sigmoid/tanh on the ACT engine, state updates on the DVE,
 - output head as 4 accumulating matmuls + bias add, then one DMA that
   replicates the (16, 4) head over the 128-wide batch shard via a
   stride-0 source dimension.
Host code only re-lays-out the tiny weights and concatenates shards.
"""

import os

import numpy as np

T = 16
HID = 512
N_CORES = 8
BSH = 128  # batch shard per core
NSPLIT = 8  # parallel DMA chunks for the Whh load

_BUILT = {}
last_results = None  # BassKernelResults of the most recent run (for tooling)


def _ensure_ntff_hook():
    """Register the axon NTFF profiling hook if the container lacks
    antenv.axon_hooks (slim boot). Mirrors trn_boot._ntff_profile_via_ctypes."""
    import contextlib
    import ctypes
    import sys
    import types

    try:
        from antenv.axon_hooks import get_axon_ntff_profile_hook  # noqa: F401

        return
    except ImportError:
        pass

    so_path = "/opt/axon/libaxon_pjrt.so"
    hook = None
    if os.path.exists(so_path):
        lib = ctypes.CDLL(so_path)
        if hasattr(lib, "axon_start_nrt_profile"):
            lib.axon_start_nrt_profile.argtypes = [
                ctypes.POINTER(ctypes.c_int64),
                ctypes.c_size_t,
            ]
            lib.axon_start_nrt_profile.restype = ctypes.c_int64
            lib.axon_stop_nrt_profile.argtypes = [ctypes.c_char_p]
            lib.axon_stop_nrt_profile.restype = ctypes.c_int64

            @contextlib.contextmanager
            def _hook(output_dir, device_ids):
                import jax

                jax.devices()  # force PJRT init so the .so's client exists
                if device_ids:
                    ids = (ctypes.c_int64 * len(device_ids))(*device_ids)
                    rc = lib.axon_start_nrt_profile(ids, len(device_ids))
                else:
                    rc = lib.axon_start_nrt_profile(None, 0)
                if rc != 0:
                    raise RuntimeError(f"axon_start_nrt_profile rc={rc}")
                try:
                    yield
                finally:
                    n = lib.axon_stop_nrt_profile(str(output_dir).encode())
                    print(f"ntff profile: {n} file(s) -> {output_dir}", file=sys.stderr)

            hook = _hook

    mod = types.ModuleType("antenv.axon_hooks")
    mod.get_axon_ntff_profile_hook = lambda: hook
    mod.set_axon_ntff_profile_hook = lambda h: None
    import antenv

    antenv.axon_hooks = mod
    sys.modules["antenv.axon_hooks"] = mod


def _build(nsteps=T):
    """Assemble the Bass module (one NeuronCore program, SPMD across 8)."""
    import concourse.bacc as bacc
    import concourse.bass as bass
    import concourse.mybir as mybir
    from concourse import tile

    f32 = mybir.dt.float32
    f16 = mybir.dt.float16
    AF = mybir.ActivationFunctionType

    nc = bacc.Bacc("TRN2", target_bir_lowering=False, debug=False)

    wT_d = nc.dram_tensor("wT", [128, 8192], f16, kind="ExternalInput")
    cst_d = nc.dram_tensor("cst", [128, 16], f32, kind="ExternalInput")
    woT_d = nc.dram_tensor("woT", [128, 16], f16, kind="ExternalInput")
    bo_d = nc.dram_tensor("bo16", [16, 4], f32, kind="ExternalInput")
    out_d = nc.dram_tensor("out", [T, BSH, 4], f32, kind="ExternalOutput")

    csz = 8192 // NSPLIT

    with tile.TileContext(nc) as tc:
        with (
            tc.tile_pool(name="w", bufs=1) as wp,
            tc.tile_pool(name="st", bufs=1) as sp,
            tc.tile_pool(name="tmp", bufs=4) as tp,
            tc.tile_pool(name="ps", bufs=2, space="PSUM") as pp,
            tc.tile_pool(name="psd", bufs=1, space="PSUM") as pdp,
        ):
            wT = wp.tile([128, 8192], f16)
            cst = sp.tile([128, 16], f32)
            woT = sp.tile([128, 16], f16)
            bo16 = sp.tile([16, 4], f32)
            for c in range(NSPLIT):
                nc.sync.dma_start(
                    wT[:, c * csz : (c + 1) * csz], wT_d[:, c * csz : (c + 1) * csz]
                )
            nc.sync.dma_start(cst[:], cst_d[:])
            nc.sync.dma_start(woT[:], woT_d[:])
            nc.sync.dma_start(bo16[:], bo_d[:])

            # "Landing" ops: give each DMA-loaded tensor a first consumer per
            # engine with no other cross-engine deps, so downstream
            # instructions carry single sync-waits (no event-semaphore
            # legalization on the hot path).
            land = tp.tile([128, 1], f32, tag="land")
            nc.vector.tensor_copy(land[:], cst[:, 0:1])
            land2 = tp.tile([16, 1], f32, tag="land2")
            nc.vector.tensor_copy(land2[:], bo16[:, 0:1])
            one16 = sp.tile([128, 1], f16)
            nc.gpsimd.memset(one16[:], 1.0)
            psd = pdp.tile([128, 1], f32, tag="dummy")
            for c in range(NSPLIT):
                nc.tensor.matmul(
                    psd[:],
                    wT[:, c * csz : c * csz + 128],
                    one16[:],
                    start=True,
                    stop=True,
                    skip_group_check=True,
                )
            psw = pdp.tile([1, 1], f32, tag="dummy2")
            nc.tensor.matmul(psw[:], one16[:], woT[:, 0:1], start=True, stop=True)

            # h_t history, fp16, column 4t+ko holds h_t[ko*128 + p]
            hs = sp.tile([128, 4 * T], f16)
            cx = sp.tile([128, 4], f32)

            def nonlin(gsrc, t):
                # gsrc [128, 16]: gate pre-activations, col blocks i|f|o|g
                sig = tp.tile([128, 12], f32, tag="sig")
                tg = tp.tile([128, 4], f32, tag="tg")
                th = tp.tile([128, 4], f32, tag="th")
                nc.scalar.activation(sig[:], gsrc[:, 0:12], AF.Sigmoid)
                nc.scalar.activation(tg[:], gsrc[:, 12:16], AF.Tanh)
                if t == 0:
                    # c starts at zero: c = sig_i * tanh_g
                    nc.vector.tensor_mul(cx[:], sig[:, 0:4], tg[:])
                else:
                    # cx *= sig_f first: only needs sig, overlaps ACT's tanh
                    t1 = tp.tile([128, 4], f32, tag="t1")
                    nc.vector.tensor_mul(cx[:], sig[:, 4:8], cx[:])
                    nc.vector.tensor_mul(t1[:], sig[:, 0:4], tg[:])
                    nc.vector.tensor_add(cx[:], cx[:], t1[:])
                nc.scalar.activation(th[:], cx[:], AF.Tanh)
                nc.vector.tensor_mul(hs[:, 4 * t : 4 * t + 4], sig[:, 8:12], th[:])

            nonlin(cst, 0)  # step 0: gates == constant, no matvec needed

            for t in range(1, nsteps):
                psg = pp.tile([128, 16], f32, tag="gates")
                for jo in range(16):
                    for ko in range(4):
                        nc.tensor.matmul(
                            psg[:, jo : jo + 1],
                            wT[:, ko * 2048 + jo * 128 : ko * 2048 + jo * 128 + 128],
                            hs[:, 4 * (t - 1) + ko : 4 * (t - 1) + ko + 1],
                            start=(ko == 0),
                            stop=(ko == 3),
                        )
                g = tp.tile([128, 16], f32, tag="g")
                nc.vector.tensor_add(g[:], psg[:], cst[:])
                nonlin(g, t)

            # head: hps[t, d] = sum_k Wo[d, k] h_t[k]
            hps = pp.tile([16, 4], f32, tag="head")
            for ko in range(4):
                nc.tensor.matmul(
                    hps[:],
                    hs[:, ko : ko + 4 * (T - 1) + 1 : 4],  # lhsT [K=128, M=16 steps]
                    woT[:, 4 * ko : 4 * ko + 4],  # rhs [K=128, N=4]
                    start=(ko == 0),
                    stop=(ko == 3),
                )
            head = sp.tile([16, 4], f32)
            nc.vector.tensor_add(head[:], hps[:], bo16[:])
            # single DMA: replicate the (16, 4) head across the 128-wide batch
            # shard via a stride-0 middle dim in the source AP
            hap = head[:]
            rep = bass.AP(hap.tensor, hap.offset, [list(hap.ap[0]), [0, BSH], [1, 4]])
            nc.sync.dma_start(out_d[:], rep)
    nc.compile()
    return nc


def prep_inputs(Whh, bih, bhh, Wo, bo):
    """Host-side weight relayout (all tensors are tiny: <5 MB total)."""
    Whh = np.asarray(Whh, np.float32)
    c = np.asarray(bih, np.float32) + np.asarray(bhh, np.float32)
    Wo = np.asarray(Wo, np.float32)
    bo = np.asarray(bo, np.float32)
    H = HID
    # reorder gate blocks from torch's i,f,g,o to i,f,o,g so sigmoid gates
    # occupy columns 0:12 and tanh gates columns 12:16
    perm = np.concatenate(
        [np.arange(0, 2 * H), np.arange(3 * H, 4 * H), np.arange(2 * H, 3 * H)]
    )
    Wp = Whh[perm]
    cp = c[perm]
    wT = np.ascontiguousarray(
        Wp.reshape(2048, 4, 128).transpose(2, 1, 0).reshape(128, 8192)
    ).astype(np.float16)
    cst = np.ascontiguousarray(cp.reshape(16, 128).T).astype(np.float32)
    woT = np.ascontiguousarray(
        Wo.reshape(4, 4, 128).transpose(2, 1, 0).reshape(128, 16)
    ).astype(np.float16)
    bo16 = np.tile(bo, (T, 1)).astype(np.float32)
    return {"wT": wT, "cst": cst, "woT": woT, "bo16": bo16}


def kernel(**inputs) -> np.ndarray:
    global last_results
    from concourse.bass_utils import run_bass_kernel_spmd

    if "nc" not in _BUILT:
        _BUILT["nc"] = _build()
    nc = _BUILT["nc"]

    in_map = prep_inputs(
        inputs["Whh"], inputs["bih"], inputs["bhh"], inputs["Wo"], inputs["bo"]
    )
    if os.environ.get("BASS_TRACE"):
        _ensure_ntff_hook()
    in_maps = [dict(in_map) for _ in range(N_CORES)]
    res = run_bass_kernel_spmd(
        nc,
        in_maps,
        core_ids=list(range(N_CORES)),
        trace=bool(os.environ.get("BASS_TRACE")),
    )
    last_results = res
    # gather: concatenate the 8 per-core batch shards
    return np.concatenate([r["out"] for r in res.results], axis=1)


# revision 11
# speedup vs baseline: 1.1942x; 1.1942x over previous
"""Trainium2 Bass kernel for nn_ESBN_77352361001553 (scatter_memory).

Math being computed (see the reference's own faithfulness note): the conv
encoder output is dead code, and the LSTM input is constant zeros, so the
gate pre-activation contribution from the input is the constant bih + bhh
for every step and every batch element. Every batch row therefore follows
the identical 16-step, 512-dim LSTM trajectory from zero state, and the
(16, 1024, 4) output is out_t = Wo @ h_t + bo broadcast across batch.

Sharding: pure data parallelism over the batch dim — each of the 8 cores
owns a 128-wide batch shard. Each core runs the recurrence on-chip:
 - gates matvec on the PE as 64 (LDWEIGHTS, MATMUL N=1) pairs per step in
   fp16 (FWL fast-weight-load path, ~27 ns/pair), accumulating the
   [128, 16] gate columns in PSUM,
 - sigmoid/tanh on the ACT engine, state updates on the DVE,
 - output head as 4 accumulating matmuls + bias add, then one DMA that
   replicates the (16, 4) head over the 128-wide batch shard via a
   stride-0 source dimension.
Host code only re-lays-out the tiny weights and concatenates shards.
"""

import os

import numpy as np

T = 16
HID = 512
N_CORES = 8
BSH = 128  # batch shard per core
NSPLIT = 8  # parallel DMA chunks for the Whh load

_BUILT = {}
last_results = None  # BassKernelResults of the most recent run (for tooling)


def _ensure_ntff_hook():
    """Register the axon NTFF profiling hook if the container lacks
    antenv.axon_hooks (slim boot). Mirrors trn_boot._ntff_profile_via_ctypes."""
    import contextlib
    import ctypes
    import sys
    import types

    try:
        from antenv.axon_hooks import get_axon_ntff_profile_hook  # noqa: F401

        return
    except ImportError:
        pass

    so_path = "/opt/axon/libaxon_pjrt.so"
    hook = None
    if os.path.exists(so_path):
        lib = ctypes.CDLL(so_path)
        if hasattr(lib, "axon_start_nrt_profile"):
            lib.axon_start_nrt_profile.argtypes = [
                ctypes.POINTER(ctypes.c_int64),
                ctypes.c_size_t,
            ]
            lib.axon_start_nrt_profile.restype = ctypes.c_int64
            lib.axon_stop_nrt_profile.argtypes = [ctypes.c_char_p]
            lib.axon_stop_nrt_profile.restype = ctypes.c_int64

            @contextlib.contextmanager
            def _hook(output_dir, device_ids):
                import jax

                jax.devices()  # force PJRT init so the .so's client exists
                if device_ids:
                    ids = (ctypes.c_int64 * len(device_ids))(*device_ids)
                    rc = lib.axon_start_nrt_profile(ids, len(device_ids))
                else:
                    rc = lib.axon_start_nrt_profile(None, 0)
                if rc != 0:
                    raise RuntimeError(f"axon_start_nrt_profile rc={rc}")
                try:
                    yield
                finally:
                    n = lib.axon_stop_nrt_profile(str(output_dir).encode())
                    print(f"ntff profile: {n} file(s) -> {output_dir}", file=sys.stderr)

            hook = _hook

    mod = types.ModuleType("antenv.axon_hooks")
    mod.get_axon_ntff_profile_hook = lambda: hook
    mod.set_axon_ntff_profile_hook = lambda h: None
    import antenv

    antenv.axon_hooks = mod
    sys.modules["antenv.axon_hooks"] = mod


def _build(nsteps=T):
    """Assemble the Bass module (one NeuronCore program, SPMD across 8)."""
    import concourse.bacc as bacc
    import concourse.bass as bass
    import concourse.mybir as mybir
    from concourse import tile

    f32 = mybir.dt.float32
    f16 = mybir.dt.float16
    AF = mybir.ActivationFunctionType

    nc = bacc.Bacc("TRN2", target_bir_lowering=False, debug=False)

    wT_d = nc.dram_tensor("wT", [128, 8192], f16, kind="ExternalInput")
    cst_d = nc.dram_tensor("cst", [128, 16], f32, kind="ExternalInput")
    woT_d = nc.dram_tensor("woT", [128, 16], f16, kind="ExternalInput")
    bo_d = nc.dram_tensor("bo16", [16, 4], f32, kind="ExternalInput")
    out_d = nc.dram_tensor("out", [T, BSH, 4], f32, kind="ExternalOutput")

    csz = 8192 // NSPLIT

    with tile.TileContext(nc) as tc:
        with (
            tc.tile_pool(name="w", bufs=1) as wp,
            tc.tile_pool(name="st", bufs=1) as sp,
            tc.tile_pool(name="tmp", bufs=4) as tp,
            tc.tile_pool(name="ps", bufs=1, space="PSUM") as pp,
            tc.tile_pool(name="psd", bufs=1, space="PSUM") as pdp,
        ):
            wT = wp.tile([128, 8192], f16)
            cst = sp.tile([128, 16], f32)
            woT = sp.tile([128, 16], f16)
            bo16 = sp.tile([16, 4], f32)
            for c in range(NSPLIT):
                nc.sync.dma_start(
                    wT[:, c * csz : (c + 1) * csz], wT_d[:, c * csz : (c + 1) * csz]
                )
            nc.sync.dma_start(cst[:], cst_d[:])
            nc.sync.dma_start(woT[:], woT_d[:])
            nc.sync.dma_start(bo16[:], bo_d[:])

            # "Landing" ops: give each DMA-loaded tensor a first consumer per
            # engine with no other cross-engine deps, so downstream
            # instructions carry single sync-waits (no event-semaphore
            # legalization on the hot path).
            land = tp.tile([128, 1], f32, tag="land")
            nc.vector.tensor_copy(land[:], cst[:, 0:1])
            land2 = tp.tile([16, 1], f32, tag="land2")
            nc.vector.tensor_copy(land2[:], bo16[:, 0:1])
            one16 = sp.tile([128, 1], f16)
            nc.gpsimd.memset(one16[:], 1.0)
            psd = pdp.tile([128, 1], f32, tag="dummy")
            for c in range(NSPLIT):
                nc.tensor.matmul(
                    psd[:],
                    wT[:, c * csz : c * csz + 128],
                    one16[:],
                    start=True,
                    stop=True,
                    skip_group_check=True,
                )
            psw = pdp.tile([1, 1], f32, tag="dummy2")
            nc.tensor.matmul(psw[:], one16[:], woT[:, 0:1], start=True, stop=True)

            # h_t history, fp16, column 4t+ko holds h_t[ko*128 + p]
            hs = sp.tile([128, 4 * T], f16)
            cx = sp.tile([128, 4], f32)

            # per-gate-group PSUM banks (column order g | f | i | o) so the
            # per-group bias-add + activation can overlap the remaining
            # groups' matmuls (no PSUM bank conflict)
            psg = [pp.tile([128, 4], f32, tag=f"ps{n}", name=f"psg{n}") for n in range(4)]

            def step0():
                tg = tp.tile([128, 4], f32, tag="tg")
                sio = tp.tile([128, 8], f32, tag="sio")
                th = tp.tile([128, 4], f32, tag="th")
                nc.scalar.activation(tg[:], cst[:, 0:4], AF.Tanh)
                nc.scalar.activation(sio[:], cst[:, 8:16], AF.Sigmoid)
                nc.vector.tensor_mul(cx[:], sio[:, 0:4], tg[:])
                nc.scalar.activation(th[:], cx[:], AF.Tanh)
                nc.vector.tensor_mul(hs[:, 0:4], sio[:, 4:8], th[:])

            step0()  # step 0: gates == constant, no matvec needed

            for t in range(1, nsteps):
                # matmuls in group order g, f, i, o; each group's bias-add and
                # activation start while later groups are still multiplying
                gadd = [tp.tile([128, 4], f32, tag=f"ga{n}", name=f"gadd{n}") for n in range(4)]
                tg = tp.tile([128, 4], f32, tag="tg")
                sf = tp.tile([128, 4], f32, tag="sf")
                si = tp.tile([128, 4], f32, tag="si")
                so = tp.tile([128, 4], f32, tag="so")
                th = tp.tile([128, 4], f32, tag="th")
                t1 = tp.tile([128, 4], f32, tag="t1")

                for gi in range(4):
                    for c in range(4):
                        jo = 4 * gi + c
                        for ko in range(4):
                            nc.tensor.matmul(
                                psg[gi][:, c : c + 1],
                                wT[
                                    :,
                                    ko * 2048
                                    + jo * 128 : ko * 2048
                                    + jo * 128
                                    + 128,
                                ],
                                hs[:, 4 * (t - 1) + ko : 4 * (t - 1) + ko + 1],
                                start=(ko == 0),
                                stop=(ko == 3),
                            )
                    nc.vector.tensor_add(
                        gadd[gi][:], psg[gi][:], cst[:, 4 * gi : 4 * gi + 4]
                    )
                    if gi == 0:
                        nc.scalar.activation(tg[:], gadd[0][:], AF.Tanh)
                    elif gi == 1:
                        nc.scalar.activation(sf[:], gadd[1][:], AF.Sigmoid)
                        nc.vector.tensor_mul(cx[:], sf[:], cx[:])
                    elif gi == 2:
                        nc.scalar.activation(si[:], gadd[2][:], AF.Sigmoid)
                        nc.vector.tensor_mul(t1[:], si[:], tg[:])
                        nc.vector.tensor_add(cx[:], cx[:], t1[:])
                        nc.scalar.activation(th[:], cx[:], AF.Tanh)
                    else:
                        nc.scalar.activation(so[:], gadd[3][:], AF.Sigmoid)
                        nc.vector.tensor_mul(
                            hs[:, 4 * t : 4 * t + 4], so[:], th[:]
                        )

            # head: hps[t, d] = sum_k Wo[d, k] h_t[k]
            hps = pdp.tile([16, 4], f32, tag="head")
            for ko in range(4):
                nc.tensor.matmul(
                    hps[:],
                    hs[:, ko : ko + 4 * (T - 1) + 1 : 4],  # lhsT [K=128, M=16 steps]
                    woT[:, 4 * ko : 4 * ko + 4],  # rhs [K=128, N=4]
                    start=(ko == 0),
                    stop=(ko == 3),
                )
            head = sp.tile([16, 4], f32)
            nc.vector.tensor_add(head[:], hps[:], bo16[:])
            # broadcast on-chip to [16, 512]: partition t holds out_t repeated
            # 128x, so the output DMA writes 16 contiguous 2 KB packets
            bc = sp.tile([16, 512], f32)
            hap = head[:]
            rep = bass.AP(hap.tensor, hap.offset, [list(hap.ap[0]), [0, BSH], [1, 4]])
            nc.vector.tensor_copy(
                bc[:].rearrange("t (b d) -> t b d", d=4), rep
            )
            nc.sync.dma_start(
                out_d.rearrange("t b d -> t (b d)"),
                bc[:],
            )
    nc.compile()
    return nc


def prep_inputs(Whh, bih, bhh, Wo, bo):
    """Host-side weight relayout (all tensors are tiny: <5 MB total)."""
    Whh = np.asarray(Whh, np.float32)
    c = np.asarray(bih, np.float32) + np.asarray(bhh, np.float32)
    Wo = np.asarray(Wo, np.float32)
    bo = np.asarray(bo, np.float32)
    H = HID
    # reorder gate blocks from torch's i,f,g,o to g,f,i,o: the g group's
    # matmuls run first so its tanh overlaps the remaining groups' matmuls
    perm = np.concatenate(
        [
            np.arange(2 * H, 3 * H),
            np.arange(H, 2 * H),
            np.arange(0, H),
            np.arange(3 * H, 4 * H),
        ]
    )
    Wp = Whh[perm]
    cp = c[perm]
    wT = np.ascontiguousarray(
        Wp.reshape(2048, 4, 128).transpose(2, 1, 0).reshape(128, 8192)
    ).astype(np.float16)
    cst = np.ascontiguousarray(cp.reshape(16, 128).T).astype(np.float32)
    woT = np.ascontiguousarray(
        Wo.reshape(4, 4, 128).transpose(2, 1, 0).reshape(128, 16)
    ).astype(np.float16)
    bo16 = np.tile(bo, (T, 1)).astype(np.float32)
    return {"wT": wT, "cst": cst, "woT": woT, "bo16": bo16}


def kernel(**inputs) -> np.ndarray:
    global last_results
    from concourse.bass_utils import run_bass_kernel_spmd

    if "nc" not in _BUILT:
        _BUILT["nc"] = _build()
    nc = _BUILT["nc"]

    in_map = prep_inputs(
        inputs["Whh"], inputs["bih"], inputs["bhh"], inputs["Wo"], inputs["bo"]
    )
    if os.environ.get("BASS_TRACE"):
        _ensure_ntff_hook()
    in_maps = [dict(in_map) for _ in range(N_CORES)]
    res = run_bass_kernel_spmd(
        nc,
        in_maps,
        core_ids=list(range(N_CORES)),
        trace=bool(os.environ.get("BASS_TRACE")),
    )
    last_results = res
    # gather: concatenate the 8 per-core batch shards
    return np.concatenate([r["out"] for r in res.results], axis=1)


# revision 15
# speedup vs baseline: 1.2250x; 1.0257x over previous
"""Trainium2 Bass kernel for nn_ESBN_77352361001553 (scatter_memory).

Math being computed (see the reference's own faithfulness note): the conv
encoder output is dead code, and the LSTM input is constant zeros, so the
gate pre-activation contribution from the input is the constant bih + bhh
for every step and every batch element. Every batch row therefore follows
the identical 16-step, 512-dim LSTM trajectory from zero state, and the
(16, 1024, 4) output is out_t = Wo @ h_t + bo broadcast across batch.

Sharding: pure data parallelism over the batch dim — each of the 8 cores
owns a 128-wide batch shard. Each core runs the recurrence on-chip:
 - gates matvec on the PE as 64 (LDWEIGHTS, MATMUL N=1) pairs per step in
   fp16 (FWL fast-weight-load path, ~27 ns/pair), accumulating the
   [128, 16] gate columns in PSUM,
 - sigmoid/tanh on the ACT engine, state updates on the DVE,
 - output head as 4 accumulating matmuls + bias add, then one DMA that
   replicates the (16, 4) head over the 128-wide batch shard via a
   stride-0 source dimension.
Host code only re-lays-out the tiny weights and concatenates shards.
"""

import os

import numpy as np

T = 16
HID = 512
N_CORES = 8
BSH = 128  # batch shard per core
NSPLIT = 8  # parallel DMA chunks for the Whh load

_BUILT = {}
last_results = None  # BassKernelResults of the most recent run (for tooling)


def _ensure_ntff_hook():
    """Register the axon NTFF profiling hook if the container lacks
    antenv.axon_hooks (slim boot). Mirrors trn_boot._ntff_profile_via_ctypes."""
    import contextlib
    import ctypes
    import sys
    import types

    try:
        from antenv.axon_hooks import get_axon_ntff_profile_hook  # noqa: F401

        return
    except ImportError:
        pass

    so_path = "/opt/axon/libaxon_pjrt.so"
    hook = None
    if os.path.exists(so_path):
        lib = ctypes.CDLL(so_path)
        if hasattr(lib, "axon_start_nrt_profile"):
            lib.axon_start_nrt_profile.argtypes = [
                ctypes.POINTER(ctypes.c_int64),
                ctypes.c_size_t,
            ]
            lib.axon_start_nrt_profile.restype = ctypes.c_int64
            lib.axon_stop_nrt_profile.argtypes = [ctypes.c_char_p]
            lib.axon_stop_nrt_profile.restype = ctypes.c_int64

            @contextlib.contextmanager
            def _hook(output_dir, device_ids):
                import jax

                jax.devices()  # force PJRT init so the .so's client exists
                if device_ids:
                    ids = (ctypes.c_int64 * len(device_ids))(*device_ids)
                    rc = lib.axon_start_nrt_profile(ids, len(device_ids))
                else:
                    rc = lib.axon_start_nrt_profile(None, 0)
                if rc != 0:
                    raise RuntimeError(f"axon_start_nrt_profile rc={rc}")
                try:
                    yield
                finally:
                    n = lib.axon_stop_nrt_profile(str(output_dir).encode())
                    print(f"ntff profile: {n} file(s) -> {output_dir}", file=sys.stderr)

            hook = _hook

    mod = types.ModuleType("antenv.axon_hooks")
    mod.get_axon_ntff_profile_hook = lambda: hook
    mod.set_axon_ntff_profile_hook = lambda h: None
    import antenv

    antenv.axon_hooks = mod
    sys.modules["antenv.axon_hooks"] = mod


def _build(nsteps=T):
    """Assemble the Bass module (one NeuronCore program, SPMD across 8)."""
    import concourse.bacc as bacc
    import concourse.bass as bass
    import concourse.mybir as mybir
    from concourse import tile

    f32 = mybir.dt.float32
    f16 = mybir.dt.float16
    AF = mybir.ActivationFunctionType

    nc = bacc.Bacc("TRN2", target_bir_lowering=False, debug=False)

    wT_d = nc.dram_tensor("wT", [128, 8192], f16, kind="ExternalInput")
    cst_d = nc.dram_tensor("cst", [128, 16], f32, kind="ExternalInput")
    woT_d = nc.dram_tensor("woT", [128, 16], f16, kind="ExternalInput")
    bo_d = nc.dram_tensor("bo16", [16, 4], f32, kind="ExternalInput")
    out_d = nc.dram_tensor("out", [T, BSH, 4], f32, kind="ExternalOutput")

    csz = 8192 // NSPLIT

    with tile.TileContext(nc) as tc:
        with (
            tc.tile_pool(name="w", bufs=1) as wp,
            tc.tile_pool(name="st", bufs=1) as sp,
            tc.tile_pool(name="tmp", bufs=4) as tp,
            tc.tile_pool(name="ps", bufs=1, space="PSUM") as pp,
            tc.tile_pool(name="psd", bufs=1, space="PSUM") as pdp,
        ):
            wT = wp.tile([128, 8192], f16)
            cst = sp.tile([128, 16], f32)
            woT = sp.tile([128, 16], f16)
            bo16 = sp.tile([16, 4], f32)
            # small tensors first (step 0 depends only on cst); the wT chunk
            # DMAs are issued from four different engine queues in parallel —
            # a single queue serializes issues at ~650 ns apiece
            nc.sync.dma_start(cst[:], cst_d[:])
            nc.sync.dma_start(woT[:], woT_d[:])
            nc.sync.dma_start(bo16[:], bo_d[:])
            dma_engines = [nc.sync, nc.gpsimd, nc.scalar]
            for c in range(NSPLIT):
                dma_engines[c % len(dma_engines)].dma_start(
                    wT[:, c * csz : (c + 1) * csz], wT_d[:, c * csz : (c + 1) * csz]
                )

            # "Landing" ops: give each DMA-loaded tensor a first consumer per
            # engine with no other cross-engine deps, so downstream
            # instructions carry single sync-waits (no event-semaphore
            # legalization on the hot path).
            land = tp.tile([128, 1], f32, tag="land")
            nc.vector.tensor_copy(land[:], cst[:, 0:1])
            land2 = tp.tile([16, 1], f32, tag="land2")
            nc.vector.tensor_copy(land2[:], bo16[:, 0:1])
            one16 = sp.tile([128, 1], f16)
            nc.gpsimd.memset(one16[:], 1.0)
            psd = pdp.tile([128, 1], f32, tag="dummy")
            for c in range(NSPLIT):
                nc.tensor.matmul(
                    psd[:],
                    wT[:, c * csz : c * csz + 128],
                    one16[:],
                    start=True,
                    stop=True,
                    skip_group_check=True,
                )
            psw = pdp.tile([1, 1], f32, tag="dummy2")
            nc.tensor.matmul(psw[:], one16[:], woT[:, 0:1], start=True, stop=True)

            # h_t history, fp16, column 4t+ko holds h_t[ko*128 + p]
            hs = sp.tile([128, 4 * T], f16)
            cx = sp.tile([128, 4], f32)

            # per-gate-group PSUM banks (column order g | f | i | o) so the
            # per-group bias-add + activation can overlap the remaining
            # groups' matmuls (no PSUM bank conflict)
            psg = [pp.tile([128, 4], f32, tag=f"ps{n}", name=f"psg{n}") for n in range(4)]

            def step0():
                tg = tp.tile([128, 4], f32, tag="tg")
                sio = tp.tile([128, 8], f32, tag="sio")
                th = tp.tile([128, 4], f32, tag="th")
                nc.scalar.activation(tg[:], cst[:, 4:8], AF.Tanh)
                nc.scalar.activation(sio[:], cst[:, 8:16], AF.Sigmoid)
                nc.vector.tensor_mul(cx[:], sio[:, 0:4], tg[:])
                nc.scalar.activation(th[:], cx[:], AF.Tanh)
                nc.vector.tensor_mul(hs[:, 0:4], sio[:, 4:8], th[:])

            step0()  # step 0: gates == constant, no matvec needed

            def mm_group(t, gi):
                for c in range(4):
                    jo = 4 * gi + c
                    for ko in range(4):
                        nc.tensor.matmul(
                            psg[gi][:, c : c + 1],
                            wT[:, ko * 2048 + jo * 128 : ko * 2048 + jo * 128 + 128],
                            hs[:, 4 * (t - 1) + ko : 4 * (t - 1) + ko + 1],
                            start=(ko == 0),
                            stop=(ko == 3),
                        )

            for t in range(1, nsteps):
                # matmuls in group order f, g, i, o (columns laid out in that
                # order); each group's bias-add + activation overlap the later
                # groups' matmuls, so only o's short chain trails the last MM
                gadd = [
                    tp.tile([128, 4], f32, tag=f"ga{n}", name=f"gadd{n}")
                    for n in range(4)
                ]
                tg = tp.tile([128, 4], f32, tag="tg")
                sf = tp.tile([128, 4], f32, tag="sf")
                si = tp.tile([128, 4], f32, tag="si")
                so = tp.tile([128, 4], f32, tag="so")
                th = tp.tile([128, 4], f32, tag="th")
                t1 = tp.tile([128, 4], f32, tag="t1")

                for gi in range(4):
                    mm_group(t, gi)
                # DVE adds + products, ordered so each runs as soon as its
                # group's stop-matmul drains
                nc.vector.tensor_add(gadd[0][:], psg[0][:], cst[:, 0:4])
                nc.scalar.activation(sf[:], gadd[0][:], AF.Sigmoid)
                nc.vector.tensor_add(gadd[1][:], psg[1][:], cst[:, 4:8])
                nc.scalar.activation(tg[:], gadd[1][:], AF.Tanh)
                nc.vector.tensor_mul(cx[:], sf[:], cx[:])
                nc.vector.tensor_add(gadd[2][:], psg[2][:], cst[:, 8:12])
                nc.scalar.activation(si[:], gadd[2][:], AF.Sigmoid)
                nc.vector.tensor_add(gadd[3][:], psg[3][:], cst[:, 12:16])
                nc.scalar.activation(so[:], gadd[3][:], AF.Sigmoid)
                nc.vector.tensor_mul(t1[:], si[:], tg[:])
                nc.vector.tensor_add(cx[:], cx[:], t1[:])
                nc.scalar.activation(th[:], cx[:], AF.Tanh)
                nc.vector.tensor_mul(hs[:, 4 * t : 4 * t + 4], so[:], th[:])

            # head: hps[t, d] = sum_k Wo[d, k] h_t[k]
            hps = pdp.tile([16, 4], f32, tag="head")
            for ko in range(4):
                nc.tensor.matmul(
                    hps[:],
                    hs[:, ko : ko + 4 * (T - 1) + 1 : 4],  # lhsT [K=128, M=16 steps]
                    woT[:, 4 * ko : 4 * ko + 4],  # rhs [K=128, N=4]
                    start=(ko == 0),
                    stop=(ko == 3),
                )
            head = sp.tile([16, 4], f32)
            nc.vector.tensor_add(head[:], hps[:], bo16[:])
            # broadcast on-chip to [16, 512]: partition t holds out_t repeated
            # 128x, so the output DMA writes 16 contiguous 2 KB packets
            bc = sp.tile([16, 512], f32)
            hap = head[:]
            rep = bass.AP(hap.tensor, hap.offset, [list(hap.ap[0]), [0, BSH], [1, 4]])
            nc.vector.tensor_copy(
                bc[:].rearrange("t (b d) -> t b d", d=4), rep
            )
            nc.sync.dma_start(
                out_d.rearrange("t b d -> t (b d)"),
                bc[:],
            )
    nc.compile()
    return nc


def prep_inputs(Whh, bih, bhh, Wo, bo):
    """Host-side weight relayout (all tensors are tiny: <5 MB total)."""
    Whh = np.asarray(Whh, np.float32)
    c = np.asarray(bih, np.float32) + np.asarray(bhh, np.float32)
    Wo = np.asarray(Wo, np.float32)
    bo = np.asarray(bo, np.float32)
    H = HID
    # reorder gate blocks from torch's i,f,g,o to f,g,i,o: earlier groups'
    # matmuls finish first, so their activations overlap later groups' matmuls
    perm = np.concatenate(
        [
            np.arange(H, 2 * H),
            np.arange(2 * H, 3 * H),
            np.arange(0, H),
            np.arange(3 * H, 4 * H),
        ]
    )
    Wp = Whh[perm]
    cp = c[perm]
    wT = np.ascontiguousarray(
        Wp.reshape(2048, 4, 128).transpose(2, 1, 0).reshape(128, 8192)
    ).astype(np.float16)
    cst = np.ascontiguousarray(cp.reshape(16, 128).T).astype(np.float32)
    woT = np.ascontiguousarray(
        Wo.reshape(4, 4, 128).transpose(2, 1, 0).reshape(128, 16)
    ).astype(np.float16)
    bo16 = np.tile(bo, (T, 1)).astype(np.float32)
    return {"wT": wT, "cst": cst, "woT": woT, "bo16": bo16}


def kernel(**inputs) -> np.ndarray:
    global last_results
    from concourse.bass_utils import run_bass_kernel_spmd

    if "nc" not in _BUILT:
        _BUILT["nc"] = _build()
    nc = _BUILT["nc"]

    in_map = prep_inputs(
        inputs["Whh"], inputs["bih"], inputs["bhh"], inputs["Wo"], inputs["bo"]
    )
    if os.environ.get("BASS_TRACE"):
        _ensure_ntff_hook()
    in_maps = [dict(in_map) for _ in range(N_CORES)]
    res = run_bass_kernel_spmd(
        nc,
        in_maps,
        core_ids=list(range(N_CORES)),
        trace=bool(os.environ.get("BASS_TRACE")),
    )
    last_results = res
    # gather: concatenate the 8 per-core batch shards
    return np.concatenate([r["out"] for r in res.results], axis=1)


# revision 17
# speedup vs baseline: 1.2275x; 1.0020x over previous
"""Trainium2 Bass kernel for nn_ESBN_77352361001553 (scatter_memory).

Math being computed (see the reference's own faithfulness note): the conv
encoder output is dead code, and the LSTM input is constant zeros, so the
gate pre-activation contribution from the input is the constant bih + bhh
for every step and every batch element. Every batch row therefore follows
the identical 16-step, 512-dim LSTM trajectory from zero state, and the
(16, 1024, 4) output is out_t = Wo @ h_t + bo broadcast across batch.

Sharding: pure data parallelism over the batch dim — each of the 8 cores
owns a 128-wide batch shard. Each core runs the recurrence on-chip:
 - gates matvec on the PE as 64 (LDWEIGHTS, MATMUL N=1) pairs per step in
   fp16 (FWL fast-weight-load path, ~27 ns/pair), accumulating the
   [128, 16] gate columns in PSUM,
 - sigmoid/tanh on the ACT engine, state updates on the DVE,
 - output head as 4 accumulating matmuls + bias add, then one DMA that
   replicates the (16, 4) head over the 128-wide batch shard via a
   stride-0 source dimension.
Host code only re-lays-out the tiny weights and concatenates shards.
"""

import os

import numpy as np

T = 16
HID = 512
N_CORES = 8
BSH = 128  # batch shard per core
NSPLIT = 6  # parallel DMA chunks for the packed weights load

_BUILT = {}
last_results = None  # BassKernelResults of the most recent run (for tooling)


def _ensure_ntff_hook():
    """Register the axon NTFF profiling hook if the container lacks
    antenv.axon_hooks (slim boot). Mirrors trn_boot._ntff_profile_via_ctypes."""
    import contextlib
    import ctypes
    import sys
    import types

    try:
        from antenv.axon_hooks import get_axon_ntff_profile_hook  # noqa: F401

        return
    except ImportError:
        pass

    so_path = "/opt/axon/libaxon_pjrt.so"
    hook = None
    if os.path.exists(so_path):
        lib = ctypes.CDLL(so_path)
        if hasattr(lib, "axon_start_nrt_profile"):
            lib.axon_start_nrt_profile.argtypes = [
                ctypes.POINTER(ctypes.c_int64),
                ctypes.c_size_t,
            ]
            lib.axon_start_nrt_profile.restype = ctypes.c_int64
            lib.axon_stop_nrt_profile.argtypes = [ctypes.c_char_p]
            lib.axon_stop_nrt_profile.restype = ctypes.c_int64

            @contextlib.contextmanager
            def _hook(output_dir, device_ids):
                import jax

                jax.devices()  # force PJRT init so the .so's client exists
                if device_ids:
                    ids = (ctypes.c_int64 * len(device_ids))(*device_ids)
                    rc = lib.axon_start_nrt_profile(ids, len(device_ids))
                else:
                    rc = lib.axon_start_nrt_profile(None, 0)
                if rc != 0:
                    raise RuntimeError(f"axon_start_nrt_profile rc={rc}")
                try:
                    yield
                finally:
                    n = lib.axon_stop_nrt_profile(str(output_dir).encode())
                    print(f"ntff profile: {n} file(s) -> {output_dir}", file=sys.stderr)

            hook = _hook

    mod = types.ModuleType("antenv.axon_hooks")
    mod.get_axon_ntff_profile_hook = lambda: hook
    mod.set_axon_ntff_profile_hook = lambda h: None
    import antenv

    antenv.axon_hooks = mod
    sys.modules["antenv.axon_hooks"] = mod


def _build(nsteps=T):
    """Assemble the Bass module (one NeuronCore program, SPMD across 8)."""
    import concourse.bacc as bacc
    import concourse.bass as bass
    import concourse.mybir as mybir
    from concourse import tile

    f32 = mybir.dt.float32
    f16 = mybir.dt.float16
    AF = mybir.ActivationFunctionType

    nc = bacc.Bacc("TRN2", target_bir_lowering=False, debug=False)

    # woT is packed into the tail columns of wT (both fp16); bo (replicated to
    # 128 rows) is packed into the tail columns of cst — 7 DMAs total
    wT_d = nc.dram_tensor("wT", [128, 8208], f16, kind="ExternalInput")
    cst_d = nc.dram_tensor("cst", [128, 20], f32, kind="ExternalInput")
    out_d = nc.dram_tensor("out", [T, BSH, 4], f32, kind="ExternalOutput")

    csz = 8208 // NSPLIT

    with tile.TileContext(nc) as tc:
        with (
            tc.tile_pool(name="w", bufs=1) as wp,
            tc.tile_pool(name="st", bufs=1) as sp,
            tc.tile_pool(name="tmp", bufs=4) as tp,
            tc.tile_pool(name="ps", bufs=1, space="PSUM") as pp,
            tc.tile_pool(name="psd", bufs=1, space="PSUM") as pdp,
        ):
            wT = wp.tile([128, 8208], f16)
            cstb = sp.tile([128, 20], f32)
            cst = cstb[:, 0:16]
            woT = wT[:, 8192:8208]
            bo16 = cstb[0:16, 16:20]
            # preload both ACT function tables while the DMAs stream in
            warm = tp.tile([1, 1], f32, tag="warm")
            nc.vector.memset(warm[:], 0.0)
            warm2 = tp.tile([1, 1], f32, tag="warm2")
            nc.scalar.activation(warm2[:], warm[:], AF.Sigmoid)
            nc.scalar.activation(warm2[:], warm[:], AF.Tanh)
            # cst first (step 0 depends only on it); wT chunks issued from
            # three engine queues in parallel — a single queue serializes
            # issues at ~650 ns apiece
            nc.sync.dma_start(cstb[:], cst_d[:])
            dma_engines = [nc.sync, nc.gpsimd, nc.scalar]
            for c in range(NSPLIT):
                dma_engines[c % len(dma_engines)].dma_start(
                    wT[:, c * csz : (c + 1) * csz], wT_d[:, c * csz : (c + 1) * csz]
                )

            # "Landing" ops: give each DMA-loaded tensor a first consumer per
            # engine with no other cross-engine deps, so downstream
            # instructions carry single sync-waits (no event-semaphore
            # legalization on the hot path).
            land = tp.tile([128, 1], f32, tag="land")
            nc.vector.tensor_copy(land[:], cstb[:, 0:1])
            one16 = sp.tile([128, 1], f16)
            nc.gpsimd.memset(one16[:], 1.0)
            psd = pdp.tile([128, 1], f32, tag="dummy")
            for c in range(NSPLIT):
                nc.tensor.matmul(
                    psd[:],
                    wT[:, c * csz : c * csz + 128],
                    one16[:],
                    start=True,
                    stop=True,
                    skip_group_check=True,
                )

            # h_t history, fp16, column 4t+ko holds h_t[ko*128 + p]
            hs = sp.tile([128, 4 * T], f16)
            cx = sp.tile([128, 4], f32)

            # per-gate-group PSUM banks (column order g | f | i | o) so the
            # per-group bias-add + activation can overlap the remaining
            # groups' matmuls (no PSUM bank conflict)
            psg = [pp.tile([128, 4], f32, tag=f"ps{n}", name=f"psg{n}") for n in range(4)]

            def step0():
                tg = tp.tile([128, 4], f32, tag="tg")
                sio = tp.tile([128, 8], f32, tag="sio")
                th = tp.tile([128, 4], f32, tag="th")
                nc.scalar.activation(tg[:], cst[:, 4:8], AF.Tanh)
                nc.scalar.activation(sio[:], cst[:, 8:16], AF.Sigmoid)
                nc.vector.tensor_mul(cx[:], sio[:, 0:4], tg[:])
                nc.scalar.activation(th[:], cx[:], AF.Tanh)
                nc.vector.tensor_mul(hs[:, 0:4], sio[:, 4:8], th[:])

            step0()  # step 0: gates == constant, no matvec needed

            def mm_group(t, gi):
                for c in range(4):
                    jo = 4 * gi + c
                    for ko in range(4):
                        nc.tensor.matmul(
                            psg[gi][:, c : c + 1],
                            wT[:, ko * 2048 + jo * 128 : ko * 2048 + jo * 128 + 128],
                            hs[:, 4 * (t - 1) + ko : 4 * (t - 1) + ko + 1],
                            start=(ko == 0),
                            stop=(ko == 3),
                        )

            for t in range(1, nsteps):
                # matmuls in group order f, g, i, o (columns laid out in that
                # order); each group's bias-add + activation overlap the later
                # groups' matmuls, so only o's short chain trails the last MM
                gadd = [
                    tp.tile([128, 4], f32, tag=f"ga{n}", name=f"gadd{n}")
                    for n in range(4)
                ]
                tg = tp.tile([128, 4], f32, tag="tg")
                sf = tp.tile([128, 4], f32, tag="sf")
                si = tp.tile([128, 4], f32, tag="si")
                so = tp.tile([128, 4], f32, tag="so")
                th = tp.tile([128, 4], f32, tag="th")
                t1 = tp.tile([128, 4], f32, tag="t1")

                for gi in range(4):
                    mm_group(t, gi)
                # DVE adds + products, ordered so each runs as soon as its
                # group's stop-matmul drains
                nc.vector.tensor_add(gadd[0][:], psg[0][:], cst[:, 0:4])
                nc.scalar.activation(sf[:], gadd[0][:], AF.Sigmoid)
                nc.vector.tensor_add(gadd[1][:], psg[1][:], cst[:, 4:8])
                nc.scalar.activation(tg[:], gadd[1][:], AF.Tanh)
                nc.vector.tensor_mul(cx[:], sf[:], cx[:])
                nc.vector.tensor_add(gadd[2][:], psg[2][:], cst[:, 8:12])
                nc.scalar.activation(si[:], gadd[2][:], AF.Sigmoid)
                nc.vector.tensor_add(gadd[3][:], psg[3][:], cst[:, 12:16])
                nc.scalar.activation(so[:], gadd[3][:], AF.Sigmoid)
                nc.vector.tensor_mul(t1[:], si[:], tg[:])
                nc.vector.tensor_add(cx[:], cx[:], t1[:])
                nc.scalar.activation(th[:], cx[:], AF.Tanh)
                nc.vector.tensor_mul(hs[:, 4 * t : 4 * t + 4], so[:], th[:])

            # head: hps[t, d] = sum_k Wo[d, k] h_t[k]
            hps = pdp.tile([16, 4], f32, tag="head")
            for ko in range(4):
                nc.tensor.matmul(
                    hps[:],
                    hs[:, ko : ko + 4 * (T - 1) + 1 : 4],  # lhsT [K=128, M=16 steps]
                    woT[:, 4 * ko : 4 * ko + 4],  # rhs [K=128, N=4]
                    start=(ko == 0),
                    stop=(ko == 3),
                )
            head = sp.tile([16, 4], f32)
            nc.vector.tensor_add(head[:], hps[:], bo16[:])
            # broadcast on-chip to [16, 512]: partition t holds out_t repeated
            # 128x, so the output DMA writes 16 contiguous 2 KB packets
            bc = sp.tile([16, 512], f32)
            hap = head[:]
            rep = bass.AP(hap.tensor, hap.offset, [list(hap.ap[0]), [0, BSH], [1, 4]])
            nc.vector.tensor_copy(
                bc[:].rearrange("t (b d) -> t b d", d=4), rep
            )
            nc.sync.dma_start(
                out_d.rearrange("t b d -> t (b d)"),
                bc[:],
            )
    nc.compile()
    return nc


def prep_inputs(Whh, bih, bhh, Wo, bo):
    """Host-side weight relayout (all tensors are tiny: <5 MB total)."""
    Whh = np.asarray(Whh, np.float32)
    c = np.asarray(bih, np.float32) + np.asarray(bhh, np.float32)
    Wo = np.asarray(Wo, np.float32)
    bo = np.asarray(bo, np.float32)
    H = HID
    # reorder gate blocks from torch's i,f,g,o to f,g,i,o: earlier groups'
    # matmuls finish first, so their activations overlap later groups' matmuls
    perm = np.concatenate(
        [
            np.arange(H, 2 * H),
            np.arange(2 * H, 3 * H),
            np.arange(0, H),
            np.arange(3 * H, 4 * H),
        ]
    )
    Wp = Whh[perm]
    cp = c[perm]
    wTm = np.ascontiguousarray(
        Wp.reshape(2048, 4, 128).transpose(2, 1, 0).reshape(128, 8192)
    ).astype(np.float16)
    woT = np.ascontiguousarray(
        Wo.reshape(4, 4, 128).transpose(2, 1, 0).reshape(128, 16)
    ).astype(np.float16)
    wT = np.concatenate([wTm, woT], axis=1)  # (128, 8208)
    cstm = np.ascontiguousarray(cp.reshape(16, 128).T).astype(np.float32)
    bo128 = np.tile(bo, (128, 1)).astype(np.float32)
    cst = np.concatenate([cstm, bo128], axis=1)  # (128, 20)
    return {"wT": wT, "cst": cst}


def kernel(**inputs) -> np.ndarray:
    global last_results
    from concourse.bass_utils import run_bass_kernel_spmd

    if "nc" not in _BUILT:
        _BUILT["nc"] = _build()
    nc = _BUILT["nc"]

    in_map = prep_inputs(
        inputs["Whh"], inputs["bih"], inputs["bhh"], inputs["Wo"], inputs["bo"]
    )
    if os.environ.get("BASS_TRACE"):
        _ensure_ntff_hook()
    in_maps = [dict(in_map) for _ in range(N_CORES)]
    res = run_bass_kernel_spmd(
        nc,
        in_maps,
        core_ids=list(range(N_CORES)),
        trace=bool(os.environ.get("BASS_TRACE")),
    )
    last_results = res
    # gather: concatenate the 8 per-core batch shards
    return np.concatenate([r["out"] for r in res.results], axis=1)
